# revision 17
# baseline (speedup 1.0000x reference)
"""Bass/Tile kernel: cosine top-20 adjacency (16384x64 embeddings) on 8 trn2 cores.

Per-core algorithm (rows sharded 2048/core via host-side input rotation so the
same SPMD graph runs on every core):
  1. Normalize rows (square -> reduce -> sqrt -> recip), fused scale-by-64 +
     fp8e4 cast (STT).  Scaled sims land at 4096*cos in PSUM.
  2. Two half-size DRAM scratches; XBAR-transpose each uint16 (fp8-pair) view
     [2048, 128] -> nt8 [128, 2048]; 4 partition-block DMAs per half assemble
     normA/normB [32, 8192] u16: partition p holds fp8 dim-pair (2p, 2p+1);
     u16 col j*2048+m of normA corresponds to emb row 4m+j (m<2048), normB
     the same for m>=2048.  Column order is a permutation, which is fine:
     the output is values-only.  Fine-grained tiles let tile-0 matmuls start
     as soon as half A is transposed.
  3. Per 128-row tile: 32 matmuls (512 cols, fp8 DoubleRowSwInterleave; PE is
     mid-pstate col-throughput-bound so dtype barely matters, fp8 halves SBUF).
     Drain: 7 groups via Act cast->bf16, 1 group fused into a DVE tensor_max
     against an already-drained group (one PSUM operand per TT is a HW limit).
     DVE pair-merges fold 7 strips -> 512 window maxima.
  4. Tail (4x max8 -> 32 candidates, 3x(max8+match_replace) -> top-24, sigmoid,
     out-DMA) is software-pipelined one tile behind the folds so the in-order
     DVE queue never idles on the serial tail chain.
  5. Self-similarity (=4096) is always the strict row max, so out[:,0] = 0 and
     out[:,1:20] = sigmoid(top24[:,1:20] / 4096) via the Act scale parameter.

NOTE: walrus --enable-ldw-opt=true (LDWEIGHTS dedup) crashes codegen in this
build; per-matmul weight loads are unavoidable.  gpsimd has no max ALU op and
DMA cannot read PSUM, so Act+DVE are the only PSUM drains.
"""

import os
import sys

import numpy as np

for _p in ("/opt/trn_rl_repo",):
    if _p not in sys.path and os.path.isdir(_p):
        sys.path.insert(0, _p)

import concourse.bass as bass  # noqa: E402
import concourse.mybir as mybir  # noqa: E402
import concourse.tile as tile  # noqa: E402
from concourse import bacc  # noqa: E402
from concourse.bass_utils import run_bass_kernel_spmd  # noqa: E402

N = 16384
D = 64
TOPK = 20
CORES = 8
R = N // CORES  # 2048 rows per core
T = R // 128  # 16 row tiles per core
G = 2048  # column group size
NEG = -1.0e30
FSCALE = 64.0  # fp8 embedding scale; sims come out x4096

f32 = mybir.dt.float32
bf16 = mybir.dt.bfloat16
fp8 = mybir.dt.float8e4
u16 = mybir.dt.uint16
AF = mybir.ActivationFunctionType
ALU = mybir.AluOpType
PM = mybir.MatmulPerfMode

V_ISSUE = 2  # issue slot drained by DVE fused tensor_max; rest: Act

_CACHE = {}


def _build_nc():
    nc = bacc.Bacc(
        "TRN2", target_bir_lowering=False, debug=False, enable_asserts=False
    )
    emb = nc.dram_tensor("embeddings", [N, D], f32, kind="ExternalInput")
    out = nc.dram_tensor("out", [R, TOPK], f32, kind="ExternalOutput")
    out_v = out[:].rearrange("(t o) k -> t o k", t=T)

    with tile.TileContext(nc) as tc:
        with tc.tile_pool(name="persist", bufs=1) as persist:
            # halves of the transposed fp8-pair matrix, partitions 0-31
            normA = persist.tile([32, N // 2], u16)
            normB = persist.tile([32, N // 2], u16)

            # ---- Prologue: normalize+scale rows, fp8 cast, XBAR transpose ----
            with (
                tc.tile_pool(name="pro_rm", bufs=1) as pro_rm,
                tc.tile_pool(name="pro_t2", bufs=1) as pro_t2,
                tc.tile_pool(name="pro_dram", bufs=1, space="DRAM") as pro_dram,
            ):
                # partition p, block a -> emb row a*128+p: chunks of the `a`
                # axis cover contiguous 2048-row blocks (scratch halves)
                emb_v = emb[:].rearrange("(a p) d -> p a d", p=128)
                rm = pro_rm.tile([128, 128, D], f32)
                sq = pro_rm.tile([128, 128, D], f32)
                ssq = pro_rm.tile([128, 128], f32)
                slen = pro_rm.tile([128, 128], f32)
                sinv = pro_rm.tile([128, 128], f32)
                rmb8 = pro_rm.tile([128, 128, D], fp8)
                scr = [
                    pro_dram.tile([N // 2, 32], u16, tag=f"scr{h}", name=f"scr{h}")
                    for h in range(2)
                ]
                engs = (nc.sync, nc.scalar)
                NCH = 8
                CW = 128 // NCH
                # all input DMAs first so nothing head-of-line blocks them
                for c in range(NCH):
                    cs = slice(c * CW, (c + 1) * CW)
                    engs[c % 2].dma_start(rm[:, cs, :], emb_v[:, cs, :])
                for c in range(NCH):
                    cs = slice(c * CW, (c + 1) * CW)
                    nc.scalar.activation(sq[:, cs, :], rm[:, cs, :], AF.Square)
                    nc.vector.tensor_reduce(
                        ssq[:, cs], sq[:, cs, :],
                        axis=mybir.AxisListType.X, op=ALU.add,
                    )
                    nc.scalar.activation(slen[:, cs], ssq[:, cs], AF.Sqrt)
                    nc.vector.reciprocal(sinv[:, cs], slen[:, cs])
                    nc.vector.scalar_tensor_tensor(
                        rmb8[:, cs, :], rm[:, cs, :], FSCALE,
                        sinv[:, cs].to_broadcast((128, CW, D)),
                        op0=ALU.mult, op1=ALU.mult,
                    )
                    # chunk c = emb rows [c*2048, (c+1)*2048) = scratch-(c//4)
                    # local rows [(c%4)*2048, (c%4+1)*2048); gpsimd SW-DGE ring
                    # so these never block input DMAs or transposes
                    sc_v = (
                        scr[c // 4][:]
                        .bitcast(fp8)
                        .rearrange("(a p) d -> p a d", p=128)
                    )
                    cl = slice((c % 4) * CW, (c % 4 + 1) * CW)
                    nc.gpsimd.dma_start(sc_v[:, cl, :], rmb8[:, cs, :])

                    if c % 4 == 3:
                        # XBAR transpose of the finished half (scalar ring;
                        # both transposes must share one ring), then 4
                        # partition-block DMAs (sync ring) assemble normA/B.
                        h = c // 4
                        sch, dst = scr[h], (normA, normB)[h]
                        sc_t = sch[:].rearrange(
                            "(m four) pd -> m (four pd)", four=4
                        )
                        nt8 = pro_t2.tile(
                            [128, N // 8], u16, tag=f"nt8{h}", name=f"nt8{h}"
                        )
                        nc.scalar.dma_start(
                            out=nt8[:], in_=sc_t[:], transpose=True
                        )
                        for j in range(4):
                            nc.sync.dma_start(
                                dst[0:32, j * 2048 : (j + 1) * 2048],
                                nt8[j * 32 : (j + 1) * 32, :],
                            )

            # fp8 pair views: [32, q, two]; col q=j*2048+m <-> emb row 4m+j
            v3A = normA[:].bitcast(fp8).rearrange("p (q two) -> p q two", two=2)
            v3B = normB[:].bitcast(fp8).rearrange("p (q two) -> p q two", two=2)

            # ---- Main loop: 16 row tiles, tail software-pipelined by 1 ----
            with (
                tc.tile_pool(name="mm_psum", bufs=2, space="PSUM") as mm_psum,
                tc.tile_pool(name="ev_cast", bufs=9) as ev_cast,
                tc.tile_pool(name="pyr", bufs=2) as pyr,
                tc.tile_pool(name="fin", bufs=3) as fin,
            ):
                pend = None  # (w0, tile_idx) awaiting tail

                def emit_tail(w0, t):
                    # candidates: top-8 of each 128-chunk of the 512 maxima
                    cand = fin.tile([128, 32], bf16, tag="cand")
                    for c in range(4):
                        nc.vector.max(
                            out=cand[:, c * 8 : (c + 1) * 8],
                            in_=w0[:, c * 128 : (c + 1) * 128],
                        )
                    top24 = fin.tile([128, 24], bf16, tag="top24")
                    cand2 = fin.tile([128, 32], bf16, tag="cand2")
                    cand3 = fin.tile([128, 32], bf16, tag="cand3")
                    nc.vector.max(out=top24[:, 0:8], in_=cand[:])
                    nc.vector.match_replace(
                        out=cand2[:], in_to_replace=top24[:, 0:8],
                        in_values=cand[:], imm_value=NEG,
                    )
                    nc.vector.max(out=top24[:, 8:16], in_=cand2[:])
                    nc.vector.match_replace(
                        out=cand3[:], in_to_replace=top24[:, 8:16],
                        in_values=cand2[:], imm_value=NEG,
                    )
                    nc.vector.max(out=top24[:, 16:24], in_=cand3[:])
                    osb = fin.tile([128, TOPK], f32, tag="osb")
                    nc.gpsimd.memset(osb[:, 0:1], 0.0)
                    nc.scalar.activation(
                        osb[:, 1:TOPK], top24[:, 1:TOPK], AF.Sigmoid,
                        scale=1.0 / (FSCALE * FSCALE),
                    )
                    nc.sync.dma_start(out_v[t], osb[:])

                for t in range(T):
                    # tile rows: emb rows x = 4*(mb*128 + (127-o)) + j
                    j, mb = t // 4, t % 4
                    lhsT = v3A[:, j * 2048 + mb * 128 : j * 2048 + mb * 128 + 128, :]

                    ca = []
                    vt = None
                    for i in range(8):
                        # issues 0-3: A-half j-blocks; 4-7: B-half
                        vv = v3A if i < 4 else v3B
                        jg = i % 4
                        ps = mm_psum.tile([128, G], f32, tag="ps")
                        for s in range(G // 512):
                            q0 = jg * G + s * 512
                            rhs = vv[:, q0 : q0 + 512, :].rearrange(
                                "p q two -> p two q"
                            )
                            nc.tensor.matmul(
                                ps[:, s * 512 : (s + 1) * 512], lhsT, rhs,
                                perf_mode=PM.DoubleRowSwInterleave,
                            )
                        if i == V_ISSUE:
                            # fused drain: max(PSUM group, drained bf16 group)
                            vt = pyr.tile([128, G], bf16, tag="vt")
                            nc.vector.tensor_max(vt[:], ps[:], ca[0][:])
                        else:
                            # two half drains: the low half's RAW dep is only
                            # the first two matmuls, so it chases them and the
                            # PSUM buffer frees ~1us earlier per group
                            cf = ev_cast.tile([128, G], bf16, tag="ca")
                            nc.scalar.activation(
                                cf[:, 0:1024], ps[:, 0:1024], AF.Copy
                            )
                            nc.scalar.activation(
                                cf[:, 1024:2048], ps[:, 1024:2048], AF.Copy
                            )
                            ca.append(cf)

                    # fold 7 strips (vt, ca1..ca6) down to 512 windows
                    p1 = pyr.tile([128, G], bf16, tag="p1")
                    nc.vector.tensor_max(p1[:], ca[1][:], ca[2][:])
                    p2 = pyr.tile([128, G], bf16, tag="p2")
                    nc.vector.tensor_max(p2[:], ca[3][:], ca[4][:])
                    p3 = pyr.tile([128, G], bf16, tag="p3")
                    nc.vector.tensor_max(p3[:], ca[5][:], ca[6][:])
                    q1 = pyr.tile([128, G], bf16, tag="q1")
                    nc.vector.tensor_max(q1[:], p1[:], p2[:])
                    q2 = pyr.tile([128, G], bf16, tag="q2")
                    nc.vector.tensor_max(q2[:], p3[:], vt[:])
                    w2 = pyr.tile([128, G], bf16, tag="w2")
                    nc.vector.tensor_max(w2[:], q1[:], q2[:])
                    w1 = pyr.tile([128, G // 2], bf16, tag="w1")
                    nc.vector.tensor_max(w1[:], w2[:, 0:1024], w2[:, 1024:2048])
                    w0 = pyr.tile([128, G // 4], bf16, tag="w0")
                    nc.vector.tensor_max(w0[:], w1[:, 0:512], w1[:, 512:1024])

                    if pend is not None:
                        emit_tail(*pend)
                    pend = (w0, t)
                emit_tail(*pend)

    nc.compile()
    return nc


def get_nc():
    if "nc" not in _CACHE:
        _CACHE["nc"] = _build_nc()
    return _CACHE["nc"]


def _row_perm():
    """perm[x] = device out index (t*128+o) holding local row x."""
    x = np.arange(R)
    j, m = x % 4, x // 4
    mb, o = m // 128, 127 - (m % 128)
    t = j * 4 + mb
    return t * 128 + o


def kernel(embeddings: np.ndarray) -> np.ndarray:
    emb = np.ascontiguousarray(np.asarray(embeddings, dtype=np.float32))
    assert emb.shape == (N, D), emb.shape
    nc = get_nc()
    in_maps = [
        {"embeddings": np.roll(emb, -i * R, axis=0)} for i in range(CORES)
    ]
    res = run_bass_kernel_spmd(nc, in_maps, core_ids=list(range(CORES)))
    _CACHE["last_results"] = res
    perm = _row_perm()
    return np.concatenate(
        [res.results[i]["out"][perm] for i in range(CORES)], axis=0
    ).astype(np.float32)


# revision 20
# speedup vs baseline: 1.0591x; 1.0591x over previous
"""Bass/Tile kernel: cosine top-20 adjacency (16384x64 embeddings) on 8 trn2 cores.

Per-core algorithm (rows sharded 2048/core via host-side input rotation so the
same SPMD graph runs on every core):
  1. Normalize rows (square -> reduce -> sqrt -> recip), fused scale-by-64 +
     fp8e4 cast (STT).  Scaled sims land at 4096*cos in PSUM.
  2. Two half-size DRAM scratches; XBAR-transpose each uint16 (fp8-pair) view
     [2048, 128] -> nt8 [128, 2048]; 4 partition-block DMAs per half assemble
     normA/normB [32, 8192] u16: partition p holds fp8 dim-pair (2p, 2p+1);
     u16 col j*2048+m of normA corresponds to emb row 4m+j (m<2048), normB
     the same for m>=2048.  Column order is a permutation, which is fine:
     the output is values-only.  Fine-grained tiles let tile-0 matmuls start
     as soon as half A is transposed.
  3. Per 128-row tile: 32 matmuls (512 cols, fp8 DoubleRowSwInterleave; PE is
     mid-pstate col-throughput-bound so dtype barely matters, fp8 halves SBUF).
     Drain: 7 groups via Act cast->bf16, 1 group fused into a DVE tensor_max
     against an already-drained group (one PSUM operand per TT is a HW limit).
     DVE pair-merges fold 7 strips -> 512 window maxima.
  4. Tail (4x max8 -> 32 candidates, 3x(max8+match_replace) -> top-24, sigmoid,
     out-DMA) is software-pipelined one tile behind the folds so the in-order
     DVE queue never idles on the serial tail chain.
  5. Self-similarity (=4096) is always the strict row max, so out[:,0] = 0 and
     out[:,1:20] = sigmoid(top24[:,1:20] / 4096) via the Act scale parameter.

NOTE: walrus --enable-ldw-opt=true (LDWEIGHTS dedup) crashes codegen in this
build; per-matmul weight loads are unavoidable.  gpsimd has no max ALU op and
DMA cannot read PSUM, so Act+DVE are the only PSUM drains.
"""

import os
import sys

import numpy as np

for _p in ("/opt/trn_rl_repo",):
    if _p not in sys.path and os.path.isdir(_p):
        sys.path.insert(0, _p)

import concourse.bass as bass  # noqa: E402
import concourse.mybir as mybir  # noqa: E402
import concourse.tile as tile  # noqa: E402
from concourse import bacc  # noqa: E402
from concourse.bass_utils import run_bass_kernel_spmd  # noqa: E402

N = 16384
D = 64
TOPK = 20
CORES = 8
R = N // CORES  # 2048 rows per core
T = R // 128  # 16 row tiles per core
G = 2048  # column group size
NEG = -1.0e30
FSCALE = 64.0  # fp8 embedding scale; sims come out x4096

f32 = mybir.dt.float32
bf16 = mybir.dt.bfloat16
fp8 = mybir.dt.float8e4
u16 = mybir.dt.uint16
AF = mybir.ActivationFunctionType
ALU = mybir.AluOpType
PM = mybir.MatmulPerfMode

V_ISSUE = 2  # issue slot drained by DVE fused tensor_max; rest: Act

_CACHE = {}


def _build_nc():
    nc = bacc.Bacc(
        "TRN2", target_bir_lowering=False, debug=False, enable_asserts=False
    )
    emb = nc.dram_tensor("embeddings", [N, D], f32, kind="ExternalInput")
    out = nc.dram_tensor("out", [R, TOPK], f32, kind="ExternalOutput")
    out_v = out[:].rearrange("(t o) k -> t o k", t=T)

    with tile.TileContext(nc) as tc:
        with tc.tile_pool(name="persist", bufs=1) as persist:
            # halves of the transposed fp8-pair matrix, partitions 0-31
            normA = persist.tile([32, N // 2], u16)
            normB = persist.tile([32, N // 2], u16)

            # ---- Prologue: normalize+scale rows, fp8 cast, XBAR transpose ----
            with (
                tc.tile_pool(name="pro_rm", bufs=1) as pro_rm,
                tc.tile_pool(name="pro_t2", bufs=1) as pro_t2,
                tc.tile_pool(name="pro_dram", bufs=1, space="DRAM") as pro_dram,
            ):
                # partition p, block a -> emb row a*128+p: chunks of the `a`
                # axis cover contiguous 2048-row blocks (scratch halves)
                emb_v = emb[:].rearrange("(a p) d -> p a d", p=128)
                rm = pro_rm.tile([128, 128, D], f32)
                sq = pro_rm.tile([128, 128, D], f32)
                ssq = pro_rm.tile([128, 128], f32)
                slen = pro_rm.tile([128, 128], f32)
                sinv = pro_rm.tile([128, 128], f32)
                rmb8 = pro_rm.tile([128, 128, D], fp8)
                scr = [
                    pro_dram.tile([N // 2, 32], u16, tag=f"scr{h}", name=f"scr{h}")
                    for h in range(2)
                ]
                engs = (nc.sync, nc.scalar)
                NCH = 8
                CW = 128 // NCH
                # all input DMAs first so nothing head-of-line blocks them
                for c in range(NCH):
                    cs = slice(c * CW, (c + 1) * CW)
                    engs[c % 2].dma_start(rm[:, cs, :], emb_v[:, cs, :])
                for c in range(NCH):
                    cs = slice(c * CW, (c + 1) * CW)
                    nc.scalar.activation(sq[:, cs, :], rm[:, cs, :], AF.Square)
                    nc.vector.tensor_reduce(
                        ssq[:, cs], sq[:, cs, :],
                        axis=mybir.AxisListType.X, op=ALU.add,
                    )
                    nc.scalar.activation(slen[:, cs], ssq[:, cs], AF.Sqrt)
                    nc.vector.reciprocal(sinv[:, cs], slen[:, cs])
                    nc.vector.scalar_tensor_tensor(
                        rmb8[:, cs, :], rm[:, cs, :], FSCALE,
                        sinv[:, cs].to_broadcast((128, CW, D)),
                        op0=ALU.mult, op1=ALU.mult,
                    )
                    # chunk c = emb rows [c*2048, (c+1)*2048) = scratch-(c//4)
                    # local rows [(c%4)*2048, (c%4+1)*2048); these are issued
                    # after all input DMAs so they can't head-of-line block
                    sc_v = (
                        scr[c // 4][:]
                        .bitcast(fp8)
                        .rearrange("(a p) d -> p a d", p=128)
                    )
                    cl = slice((c % 4) * CW, (c % 4 + 1) * CW)
                    engs[c % 2].dma_start(sc_v[:, cl, :], rmb8[:, cs, :])

                    if c % 4 == 3:
                        # XBAR transpose of the finished half (scalar ring;
                        # both transposes must share one ring), then 4
                        # partition-block DMAs (sync ring) assemble normA/B.
                        h = c // 4
                        sch, dst = scr[h], (normA, normB)[h]
                        sc_t = sch[:].rearrange(
                            "(m four) pd -> m (four pd)", four=4
                        )
                        nt8 = pro_t2.tile(
                            [128, N // 8], u16, tag=f"nt8{h}", name=f"nt8{h}"
                        )
                        nc.scalar.dma_start(
                            out=nt8[:], in_=sc_t[:], transpose=True
                        )
                        for j in range(4):
                            nc.sync.dma_start(
                                dst[0:32, j * 2048 : (j + 1) * 2048],
                                nt8[j * 32 : (j + 1) * 32, :],
                            )

            # fp8 pair views: [32, q, two]; col q=j*2048+m <-> emb row 4m+j
            v3A = normA[:].bitcast(fp8).rearrange("p (q two) -> p q two", two=2)
            v3B = normB[:].bitcast(fp8).rearrange("p (q two) -> p q two", two=2)

            # ---- Main loop: 16 row tiles, tail software-pipelined by 1 ----
            with (
                tc.tile_pool(name="mm_psum", bufs=2, space="PSUM") as mm_psum,
                tc.tile_pool(name="ev_cast", bufs=9) as ev_cast,
                tc.tile_pool(name="pyr", bufs=2) as pyr,
                tc.tile_pool(name="fin", bufs=3) as fin,
            ):
                pend = None  # (w0, tile_idx) awaiting tail

                def emit_tail(w0, t):
                    # candidates: top-8 of each 128-chunk of the 512 maxima
                    cand = fin.tile([128, 32], bf16, tag="cand")
                    for c in range(4):
                        nc.vector.max(
                            out=cand[:, c * 8 : (c + 1) * 8],
                            in_=w0[:, c * 128 : (c + 1) * 128],
                        )
                    top24 = fin.tile([128, 24], bf16, tag="top24")
                    cand2 = fin.tile([128, 32], bf16, tag="cand2")
                    cand3 = fin.tile([128, 32], bf16, tag="cand3")
                    nc.vector.max(out=top24[:, 0:8], in_=cand[:])
                    nc.vector.match_replace(
                        out=cand2[:], in_to_replace=top24[:, 0:8],
                        in_values=cand[:], imm_value=NEG,
                    )
                    nc.vector.max(out=top24[:, 8:16], in_=cand2[:])
                    nc.vector.match_replace(
                        out=cand3[:], in_to_replace=top24[:, 8:16],
                        in_values=cand2[:], imm_value=NEG,
                    )
                    nc.vector.max(out=top24[:, 16:24], in_=cand3[:])
                    osb = fin.tile([128, TOPK], f32, tag="osb")
                    nc.gpsimd.memset(osb[:, 0:1], 0.0)
                    nc.scalar.activation(
                        osb[:, 1:TOPK], top24[:, 1:TOPK], AF.Sigmoid,
                        scale=1.0 / (FSCALE * FSCALE),
                    )
                    nc.sync.dma_start(out_v[t], osb[:])

                for t in range(T):
                    # tile rows: emb rows x = 4*(mb*128 + (127-o)) + j
                    j, mb = t // 4, t % 4
                    lhsT = v3A[:, j * 2048 + mb * 128 : j * 2048 + mb * 128 + 128, :]

                    ca = []
                    vt = None
                    for i in range(8):
                        # issues 0-3: A-half j-blocks; 4-7: B-half
                        vv = v3A if i < 4 else v3B
                        jg = i % 4
                        ps = mm_psum.tile([128, G], f32, tag="ps")
                        for s in range(G // 512):
                            q0 = jg * G + s * 512
                            rhs = vv[:, q0 : q0 + 512, :].rearrange(
                                "p q two -> p two q"
                            )
                            nc.tensor.matmul(
                                ps[:, s * 512 : (s + 1) * 512], lhsT, rhs,
                                perf_mode=PM.DoubleRowSwInterleave,
                            )
                        if i == V_ISSUE:
                            # fused drain: max(PSUM group, drained bf16 group)
                            vt = pyr.tile([128, G], bf16, tag="vt")
                            nc.vector.tensor_max(vt[:], ps[:], ca[0][:])
                        else:
                            cf = ev_cast.tile([128, G], bf16, tag="ca")
                            nc.scalar.activation(cf[:], ps[:], AF.Copy)
                            ca.append(cf)

                    # fold 7 strips (vt, ca1..ca6) down to 512 windows
                    p1 = pyr.tile([128, G], bf16, tag="p1")
                    nc.vector.tensor_max(p1[:], ca[1][:], ca[2][:])
                    p2 = pyr.tile([128, G], bf16, tag="p2")
                    nc.vector.tensor_max(p2[:], ca[3][:], ca[4][:])
                    p3 = pyr.tile([128, G], bf16, tag="p3")
                    nc.vector.tensor_max(p3[:], ca[5][:], ca[6][:])
                    q1 = pyr.tile([128, G], bf16, tag="q1")
                    nc.vector.tensor_max(q1[:], p1[:], p2[:])
                    q2 = pyr.tile([128, G], bf16, tag="q2")
                    nc.vector.tensor_max(q2[:], p3[:], vt[:])
                    w2 = pyr.tile([128, G], bf16, tag="w2")
                    nc.vector.tensor_max(w2[:], q1[:], q2[:])
                    w1 = pyr.tile([128, G // 2], bf16, tag="w1")
                    nc.vector.tensor_max(w1[:], w2[:, 0:1024], w2[:, 1024:2048])
                    w0 = pyr.tile([128, G // 4], bf16, tag="w0")
                    nc.vector.tensor_max(w0[:], w1[:, 0:512], w1[:, 512:1024])

                    emit_tail(w0, t)
                del pend

    nc.compile()
    return nc


def get_nc():
    if "nc" not in _CACHE:
        _CACHE["nc"] = _build_nc()
    return _CACHE["nc"]


def _row_perm():
    """perm[x] = device out index (t*128+o) holding local row x."""
    x = np.arange(R)
    j, m = x % 4, x // 4
    mb, o = m // 128, 127 - (m % 128)
    t = j * 4 + mb
    return t * 128 + o


def kernel(embeddings: np.ndarray) -> np.ndarray:
    emb = np.ascontiguousarray(np.asarray(embeddings, dtype=np.float32))
    assert emb.shape == (N, D), emb.shape
    nc = get_nc()
    in_maps = [
        {"embeddings": np.roll(emb, -i * R, axis=0)} for i in range(CORES)
    ]
    res = run_bass_kernel_spmd(nc, in_maps, core_ids=list(range(CORES)))
    _CACHE["last_results"] = res
    perm = _row_perm()
    return np.concatenate(
        [res.results[i]["out"][perm] for i in range(CORES)], axis=0
    ).astype(np.float32)


# revision 21
# speedup vs baseline: 1.3263x; 1.2522x over previous
"""Bass/Tile kernel: cosine top-20 adjacency (16384x64 embeddings) on 8 trn2 cores.

Per-core algorithm (rows sharded 2048/core via host-side input rotation, so the
same SPMD graph runs on every core):
  1. Load embeddings row-major, compute row norms (square -> windowed reduce ->
     sqrt -> reciprocal), fused normalize+bf16-cast.
  2. Round-trip through DRAM and XBAR-transpose the [8192, 128] bf16 view ->
     normT [64, 16384] with columns permuted to [even rows | odd rows]
     (column order is irrelevant: the output is values-only).
  3. Per 128-row tile (t<8: even local rows of band t; t>=8: odd rows):
     sim = lhsT.T @ normT (bf16 matmuls into PSUM, 4 matmuls per 2048-col
     group, PSUM double-buffered). Act casts each group to bf16 (the only
     engine that can evacuate PSUM without starving the fold pipeline),
     DVE tensor_max folds halves at its 2x bf16 mode, then a 4-level fold
     pyramid to 512 windowed maxima, max8 per 128-chunk -> 32 candidates,
     3x(max8 + match_replace) -> top-24 descending.
     NOTE: both XBAR transposes must stay on ONE DMA ring - running them
     concurrently on different rings silently corrupts the transpose.
  4. Self-similarity (~1.0) is always the strict row max, so
     out[:,0] = 0 and out[:,1:20] = sigmoid(top24[:,1:20]).
"""

import os
import sys

import numpy as np

for _p in ("/opt/trn_rl_repo",):
    if _p not in sys.path and os.path.isdir(_p):
        sys.path.insert(0, _p)

import concourse.bass as bass  # noqa: E402
import concourse.mybir as mybir  # noqa: E402
import concourse.tile as tile  # noqa: E402
from concourse import bacc  # noqa: E402
from concourse.bass_utils import run_bass_kernel_spmd  # noqa: E402

N = 16384
D = 64
TOPK = 20
CORES = 8
R = N // CORES  # 2048 rows per core
T = R // 128  # 16 row tiles per core
G = 2048  # column group size
NG = N // G  # 8 column groups
H = N // 2  # even/odd half size in permuted column space
NEG = -1.0e30

f32 = mybir.dt.float32
bf16 = mybir.dt.bfloat16
AF = mybir.ActivationFunctionType
ALU = mybir.AluOpType

# Per-group evacuation: "A" = Act casts all 2048 to bf16, DVE folds at 2x.
# "S" = DVE copies the lo 1024 while Act casts the hi 1024 (independent PSUM
# bank releases), DVE folds the two SBUF halves afterwards.
GROUP_KIND = ("A", "A", "A", "A", "A", "A", "A", "A")

_CACHE = {}


def _build_nc():
    nc = bacc.Bacc(
        "TRN2", target_bir_lowering=False, debug=False, enable_asserts=False
    )
    emb = nc.dram_tensor("embeddings", [N, D], f32, kind="ExternalInput")
    out = nc.dram_tensor("out", [R, TOPK], f32, kind="ExternalOutput")
    # lhsT for tile t is a contiguous permuted-column slice; tile t<8 covers
    # even local rows 2*(t*128+q), tile t>=8 covers odd rows 2*((t-8)*128+q)+1.
    # This view un-permutes on the output DMA: out_v[h, j] = local row 2j+h.
    out_v = out[:].rearrange("(j two) k -> two j k", two=2)

    with tile.TileContext(nc) as tc:
        with tc.tile_pool(name="persist", bufs=1) as persist:
            normT = persist.tile([D, N], bf16)

            # ---- Prologue: normalize rows, cast bf16, XBAR transpose ----
            # Split into halves so the stages pipeline.
            with (
                tc.tile_pool(name="pro_rm", bufs=1) as pro_rm,
                tc.tile_pool(name="pro_t2", bufs=1) as pro_t2,
                tc.tile_pool(name="pro_dram", bufs=1, space="DRAM") as pro_dram,
            ):
                # flat [128, 128, 64] staging view: row r = p*128 + a
                emb_v = emb[:].rearrange("(p a) d -> p a d", p=128)
                rm = pro_rm.tile([128, 128, D], f32)
                sq = pro_rm.tile([128, 128, D], f32)
                ssq = pro_rm.tile([128, 128], f32)
                slen = pro_rm.tile([128, 128], f32)
                sinv = pro_rm.tile([128, 128], f32)
                rmb = pro_rm.tile([128, 128, D], bf16)
                scratch = pro_dram.tile([N, D], bf16)
                sc_v = scratch[:].rearrange("(p a) d -> p a d", p=128)
                engs = (nc.sync, nc.scalar, nc.sync, nc.scalar)
                NCH = 4
                CW = 128 // NCH
                for c in range(NCH):
                    cs = slice(c * CW, (c + 1) * CW)
                    engs[c].dma_start(rm[:, cs, :], emb_v[:, cs, :])
                    nc.scalar.activation(sq[:, cs, :], rm[:, cs, :], AF.Square)
                    nc.vector.tensor_reduce(
                        ssq[:, cs], sq[:, cs, :],
                        axis=mybir.AxisListType.X, op=ALU.add,
                    )
                    nc.scalar.activation(slen[:, cs], ssq[:, cs], AF.Sqrt)
                    nc.vector.reciprocal(sinv[:, cs], slen[:, cs])
                    nc.vector.scalar_tensor_tensor(
                        rmb[:, cs, :], rm[:, cs, :], 1.0,
                        sinv[:, cs].to_broadcast((128, CW, D)),
                        op0=ALU.mult, op1=ALU.mult,
                    )
                    engs[c].dma_start(sc_v[:, cs, :], rmb[:, cs, :])

                # XBAR transpose of the [8192, 128] bf16 view, in two row
                # chunks, BOTH on the scalar ring (concurrent transposes on
                # two rings corrupt the XBAR): nt2 partition c<64 holds
                # column c over even rows, 64+d over odd rows.
                sc_t = scratch[:].rearrange("(m two) d -> m (two d)", two=2)
                nt2 = pro_t2.tile([128, H], bf16)
                for c in range(2):
                    ms = slice(c * 4096, (c + 1) * 4096)
                    nc.scalar.dma_start(
                        out=nt2[:, ms], in_=sc_t[ms, :], transpose=True
                    )
                    nc.vector.tensor_copy(
                        normT[:, c * 4096 : (c + 1) * 4096], nt2[0:D, ms]
                    )
                    nc.sync.dma_start(
                        normT[:, H + c * 4096 : H + (c + 1) * 4096],
                        nt2[D:128, ms],
                    )

            # ---- Main loop: 16 row tiles ----
            with (
                tc.tile_pool(name="mm_psum", bufs=2, space="PSUM") as mm_psum,
                tc.tile_pool(name="ev_cast", bufs=9) as ev_cast,
                tc.tile_pool(name="ev_ch", bufs=5) as ev_ch,
                tc.tile_pool(name="pyr", bufs=2) as pyr,
                tc.tile_pool(name="fin", bufs=2) as fin,
            ):
                for t in range(T):
                    # tile t<8: even local rows; t>=8: odd local rows
                    c0 = t * 128 if t < 8 else H + (t - 8) * 128
                    lhsT = normT[:, c0 : c0 + 128]
                    l1b = pyr.tile([128, NG, G // 2], bf16, tag="l1b")
                    NM = GROUP_KIND.count("M")
                    cand = fin.tile([128, 32 + 8 * NM], bf16, tag="cand")
                    for g in range(NG):
                        if GROUP_KIND[g] == "M":
                            nc.gpsimd.memset(l1b[:, g, :], NEG)
                        ps = mm_psum.tile([128, G], f32, tag="ps")
                        for s in range(G // 512):
                            cs = slice(g * G + s * 512, g * G + (s + 1) * 512)
                            nc.tensor.matmul(
                                ps[:, s * 512 : (s + 1) * 512],
                                lhsT,
                                normT[:, cs],
                            )
                        if GROUP_KIND[g] == "M":
                            nc.vector.max(
                                out=cand[:, 32:40], in_=ps[:]
                            )
                        elif GROUP_KIND[g] == "C":
                            cf = ev_cast.tile([128, G], bf16, tag="cf")
                            nc.vector.tensor_copy(cf[:], ps[:])
                            nc.vector.tensor_max(
                                l1b[:, g, :],
                                cf[:, 0 : G // 2], cf[:, G // 2 : G],
                            )
                        elif GROUP_KIND[g] == "S":
                            ch = ev_ch.tile([128, G // 2], bf16, tag="ch")
                            nc.scalar.activation(
                                ch[:], ps[:, G // 2 : G], AF.Copy
                            )
                            cl = ev_ch.tile([128, G // 2], bf16, tag="cl")
                            nc.vector.tensor_copy(cl[:], ps[:, 0 : G // 2])
                            nc.vector.tensor_max(l1b[:, g, :], cl[:], ch[:])
                        else:
                            ca = ev_cast.tile([128, G], bf16, tag="ca")
                            nc.scalar.activation(ca[:], ps[:], AF.Copy)
                            nc.vector.tensor_max(
                                l1b[:, g, :],
                                ca[:, 0 : G // 2], ca[:, G // 2 : G],
                            )

                    # fold pyramid: 8x1024 -> 4096 -> 2048 -> 1024 -> 512
                    f2 = pyr.tile([128, 4, G // 2], bf16, tag="f2")
                    nc.vector.tensor_max(f2[:], l1b[:, 0:4, :], l1b[:, 4:8, :])
                    f3 = pyr.tile([128, 2, G // 2], bf16, tag="f3")
                    nc.vector.tensor_max(f3[:], f2[:, 0:2, :], f2[:, 2:4, :])
                    f4 = pyr.tile([128, G // 2], bf16, tag="f4")
                    nc.vector.tensor_max(f4[:], f3[:, 0, :], f3[:, 1, :])
                    f5 = pyr.tile([128, G // 4], bf16, tag="f5")
                    nc.vector.tensor_max(
                        f5[:], f4[:, 0 : G // 4], f4[:, G // 4 : G // 2]
                    )

                    # candidates: top-8 of each 128-chunk of the 512 maxima
                    for c in range(4):
                        nc.vector.max(
                            out=cand[:, c * 8 : (c + 1) * 8],
                            in_=f5[:, c * 128 : (c + 1) * 128],
                        )
                    # top-24 via 3x max8 + 2x match_replace
                    top24 = fin.tile([128, 24], bf16, tag="top24")
                    cand2 = fin.tile([128, 32 + 8 * NM], bf16, tag="cand2")
                    cand3 = fin.tile([128, 32 + 8 * NM], bf16, tag="cand3")
                    nc.vector.max(out=top24[:, 0:8], in_=cand[:])
                    nc.vector.match_replace(
                        out=cand2[:], in_to_replace=top24[:, 0:8],
                        in_values=cand[:], imm_value=NEG,
                    )
                    nc.vector.max(out=top24[:, 8:16], in_=cand2[:])
                    nc.vector.match_replace(
                        out=cand3[:], in_to_replace=top24[:, 8:16],
                        in_values=cand2[:], imm_value=NEG,
                    )
                    nc.vector.max(out=top24[:, 16:24], in_=cand3[:])

                    # epilogue: out[:,0] = 0; out[:,1:20] = sigmoid(top24[:,1:20])
                    osb = fin.tile([128, TOPK], f32, tag="osb")
                    nc.gpsimd.memset(osb[:, 0:1], 0.0)
                    nc.scalar.activation(
                        osb[:, 1:TOPK], top24[:, 1:TOPK], AF.Sigmoid
                    )
                    hh, band = (0, t) if t < 8 else (1, t - 8)
                    nc.sync.dma_start(
                        out_v[hh, band * 128 : (band + 1) * 128, :], osb[:]
                    )

    nc.compile()
    return nc


def get_nc():
    if "nc" not in _CACHE:
        _CACHE["nc"] = _build_nc()
    return _CACHE["nc"]


def kernel(embeddings: np.ndarray) -> np.ndarray:
    emb = np.ascontiguousarray(np.asarray(embeddings, dtype=np.float32))
    assert emb.shape == (N, D), emb.shape
    nc = get_nc()
    in_maps = [
        {"embeddings": np.roll(emb, -i * R, axis=0)} for i in range(CORES)
    ]
    res = run_bass_kernel_spmd(nc, in_maps, core_ids=list(range(CORES)))
    _CACHE["last_results"] = res
    return np.concatenate(
        [res.results[i]["out"] for i in range(CORES)], axis=0
    ).astype(np.float32)

